# revision 16
# baseline (speedup 1.0000x reference)
"""Trainium2 Bass kernel for nn_AttentionBlock (B=4, C=256, H=W=64).

Reference computation:
    q = Wq @ x + bq          [B, 32, N]    (N = H*W = 4096)
    k = Wk @ x + bk          [B, 32, N]
    v = Wv @ x + bv          [B, 256, N]
    scores = q^T k           [B, N, N]
    attn = softmax(scores, axis=-1)
    out = v @ attn^T + x     [B, 256, N]

Sharding: 8 cores = 4 batches x 2 query-row halves (2048 rows each).
Each core computes its (b, half) slab fully independently (no collectives).

Per-core algorithm (fp16 operands on the QK path, bf16 on the PV path —
fp16/bf16 stream the PE at 1 col/cycle and enable fast weight load, vs
~2x slower fp32; numpy-simulated end-to-end max-rel-err is ~3e-3):
  - k_rep/q_rep: fp16 projections with 4x partition replication via PE
    row-tiling (QK contraction dim is d=32, so 4 row-tiled QK matmuls run
    concurrently on the 128x128 PE array). Single-pass fp16 QK — no hi/lo
    compensation terms.
  - vT_aug[n, c]: V projection computed directly transposed (x strips as
    PE weights — fp16 enables FWL so the per-tile weight reload is cheap),
    with a ones column appended; the PV matmul then produces the softmax
    denominator as output column 256 for free. ScalarE (idle during the
    projection phase) does the PSUM->SBUF vt copies.
  - scoresT[j, i] layout means softmax never needs a free-dim reduction:
    exp() is elementwise on ScalarE over [128, 1024] double-bank PSUM
    tiles (fewer, larger activates amortize the ~230ns fixed cost), output
    in bf16 (exp range needs the fp32 exponent; fp16 would overflow).
  - out_unnorm[i, 258] accumulates over j in PSUM; normalize = per-row
    reciprocal + scale on VectorE; residual (+bv) is added from a
    host-pretransposed [i, c] tensor, so the kernel does zero transposes.
"""

import hashlib
import os
import tempfile

import numpy as np
import ml_dtypes

import concourse.bacc as bacc
import concourse.mybir as mybir
from concourse.tile import TileContext
from concourse.bass_utils import run_bass_kernel_spmd

F32 = mybir.dt.float32
F16 = mybir.dt.float16
BF16 = mybir.dt.bfloat16

P = 128          # SBUF partitions
C = 256          # channels
CT = C // P      # 2 channel tiles
N = 4096         # sequence (H*W)
NQ = N // 2      # query rows per core
D = 32           # q/k dim (C/8)
CH = 512         # query-chunk (PSUM free dim)
NCH = NQ // CH   # 4 chunks
JT = N // P      # 32 key tiles
SUBS = CH // P   # 4 i-subtiles per chunk
NROW = 4         # QK row-tiling factor (128/D)
NG = JT // NROW  # 8 jt-groups per chunk
NSLAB = 8        # xf DMA slabs (overlap load with kproj)

_NC_CACHE = []
_NEFF_CACHE = {}
_ONES = np.ones((P, JT, 2), ml_dtypes.bfloat16)


def _precompile(nc):
    """Compile the NEFF at plain-Python level (the in-jax XLA-callback
    compile path deadlocks on large fp32 kernels) and serve it to
    bass2jax's neuronx_cc_hook from a cache keyed on the BIR bytes."""
    from concourse.bass_utils import compile_bass_kernel
    bir = nc.to_json_bytes()
    key = hashlib.sha256(bir).hexdigest()
    if key not in _NEFF_CACHE:
        td = tempfile.mkdtemp(prefix="kneff_")
        neff = compile_bass_kernel(nc, td)
        with open(neff, "rb") as f:
            _NEFF_CACHE[key] = f.read()

    import concourse.bass2jax as b2j
    if not getattr(b2j, "_attn_kernel_neff_patch", False):
        orig = b2j.compile_bir_kernel

        def patched(bir_json, tmpdir, neff_name="file.neff"):
            bj = bir_json if isinstance(bir_json, bytes) else bir_json.encode()
            data = _NEFF_CACHE.get(hashlib.sha256(bj).hexdigest())
            if data is not None:
                p = os.path.join(tmpdir, neff_name)
                with open(p, "wb") as f:
                    f.write(data)
                return p
            return orig(bir_json, tmpdir, neff_name)

        b2j.compile_bir_kernel = patched
        b2j._attn_kernel_neff_patch = True


def _build():
    nc = bacc.Bacc("TRN2", target_bir_lowering=False)

    xf = nc.dram_tensor("xf", [P, CT, N], F16, kind="ExternalInput")
    qx = nc.dram_tensor("qx", [P, CT, NQ], F16, kind="ExternalInput")
    resid = nc.dram_tensor("resid", [P, NQ // P, C], F16, kind="ExternalInput")
    wqt = nc.dram_tensor("wqt", [P, CT, P], F16, kind="ExternalInput")
    wkt = nc.dram_tensor("wkt", [P, CT, P], F16, kind="ExternalInput")
    wvt = nc.dram_tensor("wvt", [P, CT, C], F16, kind="ExternalInput")
    bqr = nc.dram_tensor("bqr", [P, 1], F32, kind="ExternalInput")
    bkr = nc.dram_tensor("bkr", [P, 1], F32, kind="ExternalInput")
    vones = nc.dram_tensor("vones", [P, JT, 2], BF16, kind="ExternalInput")
    y = nc.dram_tensor("y", [NQ // P, P, C], F32, kind="ExternalOutput")
    DEBUG = bool(int(os.environ.get("ATTN_KERNEL_DEBUG", "0")))
    if DEBUG:
        dbg_k = nc.dram_tensor("dbg_k", [P, N // CH, CH], F16, kind="ExternalOutput")
        dbg_q = nc.dram_tensor("dbg_q", [P, NCH, CH], F16, kind="ExternalOutput")
        dbg_vt = nc.dram_tensor("dbg_vt", [P, JT, C + 2], BF16, kind="ExternalOutput")
        dbg_et = nc.dram_tensor("dbg_et", [P, 2, CH], BF16, kind="ExternalOutput")
        dbg_et2 = nc.dram_tensor("dbg_et2", [P, 2, CH], BF16, kind="ExternalOutput")
        dbg_pv = nc.dram_tensor("dbg_pv", [P, C + 2], F32, kind="ExternalOutput")

    ADD = mybir.AluOpType.add
    MUL = mybir.AluOpType.mult
    EXP = mybir.ActivationFunctionType.Exp

    with TileContext(nc) as tc:
        with (
            tc.tile_pool(name="const", bufs=1) as cp,
            tc.tile_pool(name="big", bufs=1) as bp,
            tc.tile_pool(name="expp", bufs=6) as ep,
            tc.tile_pool(name="yp", bufs=4) as yp,
            tc.tile_pool(name="rcpp", bufs=4) as rp,
            tc.tile_pool(name="psA", bufs=2, space="PSUM") as psA,
            tc.tile_pool(name="psPV", bufs=1, space="PSUM") as psPV,
        ):
            wq_sb = cp.tile([P, CT, P], F16, tag="wq")
            wk_sb = cp.tile([P, CT, P], F16, tag="wk")
            wv_sb = cp.tile([P, CT, C], F16, tag="wv")
            bq_sb = cp.tile([P, 1], F32, tag="bq")
            bk_sb = cp.tile([P, 1], F32, tag="bk")
            xf_sb = bp.tile([P, CT, N], F16, tag="xf")
            qx_sb = bp.tile([P, CT, NQ], F16, tag="qx")
            re_sb = bp.tile([P, NQ // P, C], F16, tag="resid")
            # k replicated 4x across partition strips; [P, 8, 512] so the
            # per-1024 projection DVE bias-cast has a matching 3D view
            krh_sb = bp.tile([P, N // CH, CH], F16, tag="krh")
            qrh_sb = bp.tile([P, NCH, CH], F16, tag="qrh")
            vt_sb = bp.tile([P, JT, C + 2], BF16, tag="vt")

            # split input DMA across two queues; xf slabs alternate and qx
            # comes in per-chunk slices so chunk 0 can start while later
            # slabs are still in flight
            nsl = N // NSLAB

            def dma_xf(sl, eng):
                eng.dma_start(
                    out=xf_sb[:, :, sl * nsl:(sl + 1) * nsl],
                    in_=xf[:, :, sl * nsl:(sl + 1) * nsl])

            def dma_qx(chn, eng):
                eng.dma_start(
                    out=qx_sb[:, :, chn * CH:(chn + 1) * CH],
                    in_=qx[:, :, chn * CH:(chn + 1) * CH])

            nc.gpsimd.dma_start(out=wk_sb, in_=wkt[:, :, :])
            nc.gpsimd.dma_start(out=bk_sb, in_=bkr[:, :])
            nc.gpsimd.dma_start(out=wq_sb, in_=wqt[:, :, :])
            nc.gpsimd.dma_start(out=wv_sb, in_=wvt[:, :, :])
            nc.gpsimd.dma_start(out=bq_sb, in_=bqr[:, :])
            dma_xf(0, nc.sync)
            dma_xf(1, nc.gpsimd)
            dma_qx(0, nc.sync)
            dma_xf(2, nc.sync)
            dma_xf(3, nc.gpsimd)
            dma_qx(1, nc.sync)
            dma_xf(4, nc.sync)
            dma_xf(5, nc.gpsimd)
            dma_xf(6, nc.sync)
            dma_xf(7, nc.gpsimd)
            dma_qx(2, nc.sync)
            dma_qx(3, nc.sync)
            # ones column for the softmax-denominator trick
            nc.gpsimd.dma_start(out=vt_sb[:, :, C:C + 2], in_=vones[:, :, :])
            nc.gpsimd.dma_start(out=re_sb, in_=resid[:, :, :])

            # ---- PE warmup: ~7.3us of dummy matmuls on a zeroed tile keep
            # the PE busy from t~7us so the HAM clock gate is guaranteed to
            # flip to 8/8 (2.4GHz; needs one fully-busy free-running 3.4us
            # window) before the real projections start. Input DMA lands
            # ~10-18us, so this costs little wall-clock and makes the whole
            # projection phase run at 2.4GHz instead of 1.2GHz.
            zw = cp.tile([P, CH], F16, tag="zwarm")
            nc.vector.memset(zw[:, :], 0)
            for w in range(9):
                ps = psA.tile([P, 2, CH], F32, tag="qk")
                for i in range(2):
                    nc.tensor.matmul(
                        ps[:, i, :], zw[:, 0:P], zw[:, :],
                        start=True, stop=True)

            # ---- projection helpers, emitted just-in-time inside chunk 0's
            # group loop so chunk 0 starts as soon as slab 0 + qx chunk 0
            # land instead of after the full projection phase.
            def kq_proj_r(w_sb, b_sb, src_sb, dst, r):
                # one 512-col region: k (or q) replicated on 4 partition
                # strips via weight col-tiling
                ps = psA.tile([P, 1, CH], F32, tag="qk")
                for ct in range(CT):
                    nc.tensor.matmul(
                        ps[:, 0, :],
                        w_sb[:, ct, :],
                        src_sb[:, ct, r * CH:(r + 1) * CH],
                        start=(ct == 0),
                        stop=(ct == CT - 1),
                    )
                nc.vector.tensor_scalar(
                    out=dst[:, r:r + 1, :], in0=ps[:, :, :],
                    scalar1=b_sb[:, :], scalar2=None, op0=ADD)

            def vproj(jg):
                ps = psA.tile([P, 4, C], F32, tag="qk")
                for jtl in range(4):
                    jt = 4 * jg + jtl
                    for ct in range(CT):
                        nc.tensor.matmul(
                            ps[:, jtl, :],
                            xf_sb[:, ct, jt * P:(jt + 1) * P],
                            wv_sb[:, ct, :],
                            start=(ct == 0),
                            stop=(ct == CT - 1),
                        )
                if jg % 2 == 0:
                    nc.scalar.copy(out=vt_sb[:, 4 * jg:4 * jg + 4, 0:C],
                                   in_=ps[:, :, :])
                else:
                    nc.vector.tensor_copy(out=vt_sb[:, 4 * jg:4 * jg + 4, 0:C],
                                          in_=ps[:, :, :])

            # ---- attention building blocks
            def qk_group(ch, g):
                # 4 row-tiled K=32 fp16 matmuls run concurrently on the
                # PE; exp over [128, 1024] (2 jt) per ACTIVATE -> bf16
                tiles = []
                for half in range(2):
                    ps = psA.tile([P, 2, CH], F32, tag="qk")
                    for i in range(2):
                        r = 2 * half + i
                        jt = NROW * g + r
                        rs = slice(32 * r, 32 * (r + 1))
                        reg, off = (jt * P) // CH, (jt * P) % CH
                        nc.tensor.matmul(
                            ps[:, i, :],
                            krh_sb[rs, reg, off:off + P],
                            qrh_sb[rs, ch, :],
                            start=True, stop=True,
                            tile_position=(32 * r, 0),
                        )
                    et = ep.tile([P, 2, CH], BF16, tag="exp")
                    nc.scalar.activation(et[:, :, :], ps[:, :, :], EXP)
                    if DEBUG and ch == 0 and g == 0:
                        nc.sync.dma_start(
                            out=(dbg_et if half == 0 else dbg_et2)[:, :, :],
                            in_=et[:, :, :])
                    tiles.append(et)
                return tiles

            def pv_group(g, tiles, pv):
                for r in range(NROW):
                    jt = NROW * g + r
                    et = tiles[r // 2]
                    for s in range(SUBS):
                        nc.tensor.matmul(
                            pv[s][:, :],
                            et[:, r % 2, s * P:(s + 1) * P],
                            vt_sb[:, jt, :],
                            start=(jt == 0),
                            stop=(jt == JT - 1),
                        )

            def normalize(ch, pv):
                for s in range(SUBS):
                    t = ch * SUBS + s
                    if DEBUG and ch == 0 and s == 0:
                        pvdbg = yp.tile([P, C + 2], F32, tag="yt", name="pvdbg")
                        nc.vector.tensor_copy(out=pvdbg, in_=pv[s][:, :])
                        nc.sync.dma_start(out=dbg_pv[:, :], in_=pvdbg)
                    rc = rp.tile([P, 1], F32, tag="rc")
                    nc.vector.reciprocal(rc[:, :], pv[s][:, C:C + 1])
                    yt = yp.tile([P, C], F32, tag="yt")
                    nc.vector.tensor_scalar(
                        out=yt[:, :], in0=pv[s][:, 0:C],
                        scalar1=rc[:, :], scalar2=None, op0=MUL)
                    nc.vector.tensor_tensor(
                        out=yt[:, :], in0=yt[:, :], in1=re_sb[:, t, :], op=ADD)
                    nc.gpsimd.dma_start(out=y[t, :, :], in_=yt[:, :])

            # prologue: just enough projections for QK(0,0) and PV(0,0),
            # then the flat loop emits the rest just-in-time
            kq_proj_r(wk_sb, bk_sb, xf_sb, krh_sb, 0)
            kq_proj_r(wk_sb, bk_sb, xf_sb, krh_sb, 1)
            kq_proj_r(wk_sb, bk_sb, xf_sb, krh_sb, 2)
            kq_proj_r(wq_sb, bq_sb, qx_sb, qrh_sb, 0)
            vproj(0)
            vproj(1)
            pvs = {0: [psPV.tile([P, C + 2], F32, tag=f"pv{s}", name=f"pv{s}")
                       for s in range(SUBS)]}
            pending = (0, 0, qk_group(0, 0))

            if DEBUG:
                nc.sync.dma_start(out=dbg_k[:, :, :], in_=krh_sb[:, :, :])
                nc.sync.dma_start(out=dbg_q[:, :, :], in_=qrh_sb[:, :, :])
                nc.sync.dma_start(out=dbg_vt[:, :, :], in_=vt_sb[:, :, :])

            # ---- attention: flat (chunk, group) loop, software-pipelined
            # one group ahead (including across chunk boundaries)
            steps = [(ch, g) for ch in range(NCH) for g in range(NG)]
            for ch, g in steps[1:] + [(None, None)]:
                if ch is not None:
                    if g == 0:
                        pvs[ch] = [psPV.tile([P, C + 2], F32, tag=f"pv{s}",
                                             name=f"pv{s}")
                                   for s in range(SUBS)]
                    tiles = qk_group(ch, g)
                pch, pg, ptiles = pending
                pv_group(pg, ptiles, pvs[pch])
                if pg == NG - 1:
                    normalize(pch, pvs.pop(pch))
                # chunk-0 JIT projections (after this group's qk/pv so the
                # psA rotation never makes attention wait on future slabs)
                if ch == 0:
                    if g + 2 < NG:
                        kq_proj_r(wk_sb, bk_sb, xf_sb, krh_sb, g + 2)
                    if g + 1 < NG:
                        vproj(g + 1)
                    if g == 4:
                        kq_proj_r(wq_sb, bq_sb, qx_sb, qrh_sb, 1)
                elif ch is not None and g == 4 and ch + 1 < NCH:
                    kq_proj_r(wq_sb, bq_sb, qx_sb, qrh_sb, ch + 1)
                pending = (ch, g, tiles) if ch is not None else None

    nc.compile()
    return nc


def make_in_maps(x, Wq, bq, Wk, bk, Wv, bv):
    x = np.ascontiguousarray(x, np.float32)
    Wq = np.asarray(Wq, np.float32)
    bq = np.asarray(bq, np.float32)
    Wk = np.asarray(Wk, np.float32)
    bk = np.asarray(bk, np.float32)
    Wv = np.asarray(Wv, np.float32)
    bv = np.asarray(bv, np.float32)

    wqt = np.ascontiguousarray(
        np.tile(Wq.T.reshape(CT, P, D).transpose(1, 0, 2), (1, 1, NROW))
    ).astype(np.float16)
    wkt = np.ascontiguousarray(
        np.tile(Wk.T.reshape(CT, P, D).transpose(1, 0, 2), (1, 1, NROW))
    ).astype(np.float16)
    wvt = np.ascontiguousarray(
        Wv.T.reshape(CT, P, C).transpose(1, 0, 2)).astype(np.float16)
    bqr = np.ascontiguousarray(np.tile(bq, NROW).reshape(P, 1))
    bkr = np.ascontiguousarray(np.tile(bk, NROW).reshape(P, 1))

    in_maps = []
    for core in range(8):
        b, h = divmod(core, 2)
        xb = x[b].reshape(C, N)
        xf_h = np.ascontiguousarray(
            xb.reshape(CT, P, N).transpose(1, 0, 2)).astype(np.float16)
        qx_h = np.ascontiguousarray(xf_h[:, :, h * NQ:(h + 1) * NQ])
        res_h = np.ascontiguousarray(
            (xb[:, h * NQ:(h + 1) * NQ].T + bv[None, :])
            .reshape(NQ // P, P, C).transpose(1, 0, 2)).astype(np.float16)
        in_maps.append({
            "xf": xf_h, "qx": qx_h, "resid": res_h,
            "wqt": wqt, "wkt": wkt, "wvt": wvt,
            "bqr": bqr, "bkr": bkr,
            "vones": _ONES,
        })
    return in_maps


def kernel(x, Wq, bq, Wk, bk, Wv, bv):
    if not _NC_CACHE:
        _NC_CACHE.append(_build())
    nc = _NC_CACHE[0]
    _precompile(nc)

    in_maps = make_in_maps(x, Wq, bq, Wk, bk, Wv, bv)
    res = run_bass_kernel_spmd(nc, in_maps, core_ids=list(range(8)))

    B = np.asarray(x).shape[0]
    out = np.empty((B, C, N), np.float32)
    for core in range(8):
        b, h = divmod(core, 2)
        slab = res.results[core]["y"].reshape(NQ, C)
        out[b, :, h * NQ:(h + 1) * NQ] = slab.T
    return out.reshape(B, C, 64, 64)


# revision 17
# speedup vs baseline: 1.0218x; 1.0218x over previous
"""Trainium2 Bass kernel for nn_AttentionBlock (B=4, C=256, H=W=64).

Reference computation:
    q = Wq @ x + bq          [B, 32, N]    (N = H*W = 4096)
    k = Wk @ x + bk          [B, 32, N]
    v = Wv @ x + bv          [B, 256, N]
    scores = q^T k           [B, N, N]
    attn = softmax(scores, axis=-1)
    out = v @ attn^T + x     [B, 256, N]

Sharding: 8 cores = 4 batches x 2 query-row halves (2048 rows each).
Each core computes its (b, half) slab fully independently (no collectives).

Per-core algorithm (fp16 operands on the QK path, bf16 on the PV path —
fp16/bf16 stream the PE at 1 col/cycle and enable fast weight load, vs
~2x slower fp32; numpy-simulated end-to-end max-rel-err is ~3e-3):
  - k_rep/q_rep: fp16 projections with 4x partition replication via PE
    row-tiling (QK contraction dim is d=32, so 4 row-tiled QK matmuls run
    concurrently on the 128x128 PE array). Single-pass fp16 QK — no hi/lo
    compensation terms.
  - vT_aug[n, c]: V projection computed directly transposed (x strips as
    PE weights — fp16 enables FWL so the per-tile weight reload is cheap),
    with a ones column appended; the PV matmul then produces the softmax
    denominator as output column 256 for free. ScalarE (idle during the
    projection phase) does the PSUM->SBUF vt copies.
  - scoresT[j, i] layout means softmax never needs a free-dim reduction:
    exp() is elementwise on ScalarE over [128, 1024] double-bank PSUM
    tiles (fewer, larger activates amortize the ~230ns fixed cost), output
    in bf16 (exp range needs the fp32 exponent; fp16 would overflow).
  - out_unnorm[i, 258] accumulates over j in PSUM; normalize = per-row
    reciprocal + scale on VectorE; residual (+bv) is added from a
    host-pretransposed [i, c] tensor, so the kernel does zero transposes.
"""

import hashlib
import os
import tempfile

import numpy as np
import ml_dtypes

import concourse.bacc as bacc
import concourse.mybir as mybir
from concourse.tile import TileContext
from concourse.bass_utils import run_bass_kernel_spmd

F32 = mybir.dt.float32
F16 = mybir.dt.float16
BF16 = mybir.dt.bfloat16

P = 128          # SBUF partitions
C = 256          # channels
CT = C // P      # 2 channel tiles
N = 4096         # sequence (H*W)
NQ = N // 2      # query rows per core
D = 32           # q/k dim (C/8)
CH = 512         # query-chunk (PSUM free dim)
NCH = NQ // CH   # 4 chunks
JT = N // P      # 32 key tiles
SUBS = CH // P   # 4 i-subtiles per chunk
NROW = 4         # QK row-tiling factor (128/D)
NG = JT // NROW  # 8 jt-groups per chunk
NSLAB = 8        # xf DMA slabs (overlap load with kproj)

_NC_CACHE = []
_NEFF_CACHE = {}
_ONES = np.ones((P, JT, 2), ml_dtypes.bfloat16)


def _precompile(nc):
    """Compile the NEFF at plain-Python level (the in-jax XLA-callback
    compile path deadlocks on large fp32 kernels) and serve it to
    bass2jax's neuronx_cc_hook from a cache keyed on the BIR bytes."""
    from concourse.bass_utils import compile_bass_kernel
    bir = nc.to_json_bytes()
    key = hashlib.sha256(bir).hexdigest()
    if key not in _NEFF_CACHE:
        td = tempfile.mkdtemp(prefix="kneff_")
        neff = compile_bass_kernel(nc, td)
        with open(neff, "rb") as f:
            _NEFF_CACHE[key] = f.read()

    import concourse.bass2jax as b2j
    if not getattr(b2j, "_attn_kernel_neff_patch", False):
        orig = b2j.compile_bir_kernel

        def patched(bir_json, tmpdir, neff_name="file.neff"):
            bj = bir_json if isinstance(bir_json, bytes) else bir_json.encode()
            data = _NEFF_CACHE.get(hashlib.sha256(bj).hexdigest())
            if data is not None:
                p = os.path.join(tmpdir, neff_name)
                with open(p, "wb") as f:
                    f.write(data)
                return p
            return orig(bir_json, tmpdir, neff_name)

        b2j.compile_bir_kernel = patched
        b2j._attn_kernel_neff_patch = True


def _build():
    nc = bacc.Bacc("TRN2", target_bir_lowering=False)

    xf = nc.dram_tensor("xf", [P, CT, N], F16, kind="ExternalInput")
    qx = nc.dram_tensor("qx", [P, CT, NQ], F16, kind="ExternalInput")
    resid = nc.dram_tensor("resid", [P, NQ // P, C], F16, kind="ExternalInput")
    wqt = nc.dram_tensor("wqt", [P, CT, P], F16, kind="ExternalInput")
    wkt = nc.dram_tensor("wkt", [P, CT, P], F16, kind="ExternalInput")
    wvt = nc.dram_tensor("wvt", [P, CT, C], F16, kind="ExternalInput")
    bqr = nc.dram_tensor("bqr", [P, 1], F32, kind="ExternalInput")
    bkr = nc.dram_tensor("bkr", [P, 1], F32, kind="ExternalInput")
    vones = nc.dram_tensor("vones", [P, JT, 2], BF16, kind="ExternalInput")
    y = nc.dram_tensor("y", [NQ // P, P, C], F32, kind="ExternalOutput")
    DEBUG = bool(int(os.environ.get("ATTN_KERNEL_DEBUG", "0")))
    if DEBUG:
        dbg_k = nc.dram_tensor("dbg_k", [P, N // CH, CH], F16, kind="ExternalOutput")
        dbg_q = nc.dram_tensor("dbg_q", [P, NCH, CH], F16, kind="ExternalOutput")
        dbg_vt = nc.dram_tensor("dbg_vt", [P, JT, C + 2], BF16, kind="ExternalOutput")
        dbg_et = nc.dram_tensor("dbg_et", [P, 2, CH], BF16, kind="ExternalOutput")
        dbg_et2 = nc.dram_tensor("dbg_et2", [P, 2, CH], BF16, kind="ExternalOutput")
        dbg_pv = nc.dram_tensor("dbg_pv", [P, C + 2], F32, kind="ExternalOutput")

    ADD = mybir.AluOpType.add
    MUL = mybir.AluOpType.mult
    EXP = mybir.ActivationFunctionType.Exp

    with TileContext(nc) as tc:
        with (
            tc.tile_pool(name="const", bufs=1) as cp,
            tc.tile_pool(name="big", bufs=1) as bp,
            tc.tile_pool(name="expp", bufs=6) as ep,
            tc.tile_pool(name="yp", bufs=4) as yp,
            tc.tile_pool(name="rcpp", bufs=4) as rp,
            tc.tile_pool(name="psA", bufs=2, space="PSUM") as psA,
            tc.tile_pool(name="psPV", bufs=1, space="PSUM") as psPV,
        ):
            wq_sb = cp.tile([P, CT, P], F16, tag="wq")
            wk_sb = cp.tile([P, CT, P], F16, tag="wk")
            wv_sb = cp.tile([P, CT, C], F16, tag="wv")
            bq_sb = cp.tile([P, 1], F32, tag="bq")
            bk_sb = cp.tile([P, 1], F32, tag="bk")
            xf_sb = bp.tile([P, CT, N], F16, tag="xf")
            qx_sb = bp.tile([P, CT, NQ], F16, tag="qx")
            re_sb = bp.tile([P, NQ // P, C], F16, tag="resid")
            # k replicated 4x across partition strips; [P, 8, 512] so the
            # per-1024 projection DVE bias-cast has a matching 3D view
            krh_sb = bp.tile([P, N // CH, CH], F16, tag="krh")
            qrh_sb = bp.tile([P, NCH, CH], F16, tag="qrh")
            vt_sb = bp.tile([P, JT, C + 2], BF16, tag="vt")

            # split input DMA across two queues; xf slabs alternate so the
            # two queues transfer in parallel and kproj is never starved
            nsl = N // NSLAB
            nc.gpsimd.dma_start(out=wk_sb, in_=wkt[:, :, :])
            nc.gpsimd.dma_start(out=bk_sb, in_=bkr[:, :])
            nc.gpsimd.dma_start(out=wv_sb, in_=wvt[:, :, :])
            for sl in range(NSLAB):
                eng = nc.sync if sl % 2 == 0 else nc.gpsimd
                eng.dma_start(
                    out=xf_sb[:, :, sl * nsl:(sl + 1) * nsl],
                    in_=xf[:, :, sl * nsl:(sl + 1) * nsl])
            nc.gpsimd.dma_start(out=wq_sb, in_=wqt[:, :, :])
            nc.gpsimd.dma_start(out=bq_sb, in_=bqr[:, :])
            # ones column for the softmax-denominator trick
            nc.gpsimd.dma_start(out=vt_sb[:, :, C:C + 2], in_=vones[:, :, :])
            nc.sync.dma_start(out=qx_sb, in_=qx[:, :, :])
            nc.gpsimd.dma_start(out=re_sb, in_=resid[:, :, :])

            # ---- PE warmup: ~7.3us of dummy matmuls on a zeroed tile keep
            # the PE busy from t~7us so the HAM clock gate is guaranteed to
            # flip to 8/8 (2.4GHz; needs one fully-busy free-running 3.4us
            # window) before the real projections start. Input DMA lands
            # ~10-18us, so this costs little wall-clock and makes the whole
            # projection phase run at 2.4GHz instead of 1.2GHz.
            zw = cp.tile([P, CH], F16, tag="zwarm")
            nc.vector.memset(zw[:, :], 0)
            for w in range(9):
                ps = psA.tile([P, 2, CH], F32, tag="qk")
                for i in range(2):
                    nc.tensor.matmul(
                        ps[:, i, :], zw[:, 0:P], zw[:, :],
                        start=True, stop=True)

            # ---- projections. kproj (k replicated on 4 partition strips
            # via weight col-tiling) is interleaved with vproj per xf slab
            # pair so the PE tracks DMA arrival; qproj (qx lands last) runs
            # at the end.
            def kq_proj(w_sb, b_sb, src_sb, dst, g):
                ps = psA.tile([P, 2, CH], F32, tag="qk")
                for i in range(2):
                    chn = 2 * g + i
                    for ct in range(CT):
                        nc.tensor.matmul(
                            ps[:, i, :],
                            w_sb[:, ct, :],
                            src_sb[:, ct, chn * CH:(chn + 1) * CH],
                            start=(ct == 0),
                            stop=(ct == CT - 1),
                        )
                nc.vector.tensor_scalar(
                    out=dst[:, 2 * g:2 * g + 2, :], in0=ps[:, :, :],
                    scalar1=b_sb[:, :], scalar2=None, op0=ADD)

            def vproj(jg):
                ps = psA.tile([P, 4, C], F32, tag="qk")
                for jtl in range(4):
                    jt = 4 * jg + jtl
                    for ct in range(CT):
                        nc.tensor.matmul(
                            ps[:, jtl, :],
                            xf_sb[:, ct, jt * P:(jt + 1) * P],
                            wv_sb[:, ct, :],
                            start=(ct == 0),
                            stop=(ct == CT - 1),
                        )
                if jg % 2 == 0:
                    nc.scalar.copy(out=vt_sb[:, 4 * jg:4 * jg + 4, 0:C],
                                   in_=ps[:, :, :])
                else:
                    nc.vector.tensor_copy(out=vt_sb[:, 4 * jg:4 * jg + 4, 0:C],
                                          in_=ps[:, :, :])

            def warm_filler():
                ps = psA.tile([P, 2, CH], F32, tag="qk")
                nc.tensor.matmul(ps[:, 0, :], zw[:, 0:P], zw[:, :],
                                 start=True, stop=True)

            for g in range(N // CH // 2):
                vproj(2 * g)
                warm_filler()
                kq_proj(wk_sb, bk_sb, xf_sb, krh_sb, g)
                warm_filler()
                vproj(2 * g + 1)
                warm_filler()
            for g in range(NQ // CH // 2):
                kq_proj(wq_sb, bq_sb, qx_sb, qrh_sb, g)

            # ---- attention building blocks
            def qk_group(ch, g):
                # 4 row-tiled K=32 fp16 matmuls run concurrently on the
                # PE; exp over [128, 1024] (2 jt) per ACTIVATE -> bf16
                tiles = []
                for half in range(2):
                    ps = psA.tile([P, 2, CH], F32, tag="qk")
                    for i in range(2):
                        r = 2 * half + i
                        jt = NROW * g + r
                        rs = slice(32 * r, 32 * (r + 1))
                        reg, off = (jt * P) // CH, (jt * P) % CH
                        nc.tensor.matmul(
                            ps[:, i, :],
                            krh_sb[rs, reg, off:off + P],
                            qrh_sb[rs, ch, :],
                            start=True, stop=True,
                            tile_position=(32 * r, 0),
                        )
                    et = ep.tile([P, 2, CH], BF16, tag="exp")
                    nc.scalar.activation(et[:, :, :], ps[:, :, :], EXP)
                    if DEBUG and ch == 0 and g == 0:
                        nc.sync.dma_start(
                            out=(dbg_et if half == 0 else dbg_et2)[:, :, :],
                            in_=et[:, :, :])
                    tiles.append(et)
                return tiles

            def pv_group(g, tiles, pv):
                for r in range(NROW):
                    jt = NROW * g + r
                    et = tiles[r // 2]
                    for s in range(SUBS):
                        nc.tensor.matmul(
                            pv[s][:, :],
                            et[:, r % 2, s * P:(s + 1) * P],
                            vt_sb[:, jt, :],
                            start=(jt == 0),
                            stop=(jt == JT - 1),
                        )

            def normalize(ch, pv):
                for s in range(SUBS):
                    t = ch * SUBS + s
                    if DEBUG and ch == 0 and s == 0:
                        pvdbg = yp.tile([P, C + 2], F32, tag="yt", name="pvdbg")
                        nc.vector.tensor_copy(out=pvdbg, in_=pv[s][:, :])
                        nc.sync.dma_start(out=dbg_pv[:, :], in_=pvdbg)
                    rc = rp.tile([P, 1], F32, tag="rc")
                    nc.vector.reciprocal(rc[:, :], pv[s][:, C:C + 1])
                    yt = yp.tile([P, C], F32, tag="yt")
                    nc.vector.tensor_scalar(
                        out=yt[:, :], in0=pv[s][:, 0:C],
                        scalar1=rc[:, :], scalar2=None, op0=MUL)
                    nc.vector.tensor_tensor(
                        out=yt[:, :], in0=yt[:, :], in1=re_sb[:, t, :], op=ADD)
                    nc.gpsimd.dma_start(out=y[t, :, :], in_=yt[:, :])

            # pre-issue QK(0,0) so ScalarE has exp work during vproj
            pvs = {0: [psPV.tile([P, C + 2], F32, tag=f"pv{s}", name=f"pv{s}")
                       for s in range(SUBS)]}
            pending = (0, 0, qk_group(0, 0))

            if DEBUG:
                nc.sync.dma_start(out=dbg_k[:, :, :], in_=krh_sb[:, :, :])
                nc.sync.dma_start(out=dbg_q[:, :, :], in_=qrh_sb[:, :, :])
                nc.sync.dma_start(out=dbg_vt[:, :, :], in_=vt_sb[:, :, :])

            # ---- attention: flat (chunk, group) loop, software-pipelined
            # one group ahead (including across chunk boundaries)
            steps = [(ch, g) for ch in range(NCH) for g in range(NG)]
            for ch, g in steps[1:] + [(None, None)]:
                if ch is not None:
                    if g == 0:
                        pvs[ch] = [psPV.tile([P, C + 2], F32, tag=f"pv{s}",
                                             name=f"pv{s}")
                                   for s in range(SUBS)]
                    tiles = qk_group(ch, g)
                pch, pg, ptiles = pending
                pv_group(pg, ptiles, pvs[pch])
                if pg == NG - 1:
                    normalize(pch, pvs.pop(pch))
                pending = (ch, g, tiles) if ch is not None else None

    nc.compile()
    return nc


def make_in_maps(x, Wq, bq, Wk, bk, Wv, bv):
    x = np.ascontiguousarray(x, np.float32)
    Wq = np.asarray(Wq, np.float32)
    bq = np.asarray(bq, np.float32)
    Wk = np.asarray(Wk, np.float32)
    bk = np.asarray(bk, np.float32)
    Wv = np.asarray(Wv, np.float32)
    bv = np.asarray(bv, np.float32)

    wqt = np.ascontiguousarray(
        np.tile(Wq.T.reshape(CT, P, D).transpose(1, 0, 2), (1, 1, NROW))
    ).astype(np.float16)
    wkt = np.ascontiguousarray(
        np.tile(Wk.T.reshape(CT, P, D).transpose(1, 0, 2), (1, 1, NROW))
    ).astype(np.float16)
    wvt = np.ascontiguousarray(
        Wv.T.reshape(CT, P, C).transpose(1, 0, 2)).astype(np.float16)
    bqr = np.ascontiguousarray(np.tile(bq, NROW).reshape(P, 1))
    bkr = np.ascontiguousarray(np.tile(bk, NROW).reshape(P, 1))

    in_maps = []
    for core in range(8):
        b, h = divmod(core, 2)
        xb = x[b].reshape(C, N)
        xf_h = np.ascontiguousarray(
            xb.reshape(CT, P, N).transpose(1, 0, 2)).astype(np.float16)
        qx_h = np.ascontiguousarray(xf_h[:, :, h * NQ:(h + 1) * NQ])
        res_h = np.ascontiguousarray(
            (xb[:, h * NQ:(h + 1) * NQ].T + bv[None, :])
            .reshape(NQ // P, P, C).transpose(1, 0, 2)).astype(np.float16)
        in_maps.append({
            "xf": xf_h, "qx": qx_h, "resid": res_h,
            "wqt": wqt, "wkt": wkt, "wvt": wvt,
            "bqr": bqr, "bkr": bkr,
            "vones": _ONES,
        })
    return in_maps


def kernel(x, Wq, bq, Wk, bk, Wv, bv):
    if not _NC_CACHE:
        _NC_CACHE.append(_build())
    nc = _NC_CACHE[0]
    _precompile(nc)

    in_maps = make_in_maps(x, Wq, bq, Wk, bk, Wv, bv)
    res = run_bass_kernel_spmd(nc, in_maps, core_ids=list(range(8)))

    B = np.asarray(x).shape[0]
    out = np.empty((B, C, N), np.float32)
    for core in range(8):
        b, h = divmod(core, 2)
        slab = res.results[core]["y"].reshape(NQ, C)
        out[b, :, h * NQ:(h + 1) * NQ] = slab.T
    return out.reshape(B, C, 64, 64)


# revision 18
# speedup vs baseline: 1.0447x; 1.0224x over previous
"""Trainium2 Bass kernel for nn_AttentionBlock (B=4, C=256, H=W=64).

Reference computation:
    q = Wq @ x + bq          [B, 32, N]    (N = H*W = 4096)
    k = Wk @ x + bk          [B, 32, N]
    v = Wv @ x + bv          [B, 256, N]
    scores = q^T k           [B, N, N]
    attn = softmax(scores, axis=-1)
    out = v @ attn^T + x     [B, 256, N]

Sharding: 8 cores = 4 batches x 2 query-row halves (2048 rows each).
Each core computes its (b, half) slab fully independently (no collectives).

Per-core algorithm (fp16 operands on the QK path, bf16 on the PV path —
fp16/bf16 stream the PE at 1 col/cycle and enable fast weight load, vs
~2x slower fp32; numpy-simulated end-to-end max-rel-err is ~3e-3):
  - k_rep/q_rep: fp16 projections with 4x partition replication via PE
    row-tiling (QK contraction dim is d=32, so 4 row-tiled QK matmuls run
    concurrently on the 128x128 PE array). Single-pass fp16 QK — no hi/lo
    compensation terms.
  - vT_aug[n, c]: V projection computed directly transposed (x strips as
    PE weights — fp16 enables FWL so the per-tile weight reload is cheap),
    with a ones column appended; the PV matmul then produces the softmax
    denominator as output column 256 for free. ScalarE (idle during the
    projection phase) does the PSUM->SBUF vt copies.
  - scoresT[j, i] layout means softmax never needs a free-dim reduction:
    exp() is elementwise on ScalarE over [128, 1024] double-bank PSUM
    tiles (fewer, larger activates amortize the ~230ns fixed cost), output
    in bf16 (exp range needs the fp32 exponent; fp16 would overflow).
  - out_unnorm[i, 258] accumulates over j in PSUM; normalize = per-row
    reciprocal + scale on VectorE; residual (+bv) is added from a
    host-pretransposed [i, c] tensor, so the kernel does zero transposes.
"""

import hashlib
import os
import tempfile

import numpy as np
import ml_dtypes

import concourse.bacc as bacc
import concourse.mybir as mybir
from concourse.tile import TileContext
from concourse.bass_utils import run_bass_kernel_spmd

F32 = mybir.dt.float32
F16 = mybir.dt.float16
BF16 = mybir.dt.bfloat16

P = 128          # SBUF partitions
C = 256          # channels
CT = C // P      # 2 channel tiles
N = 4096         # sequence (H*W)
NQ = N // 2      # query rows per core
D = 32           # q/k dim (C/8)
CH = 512         # query-chunk (PSUM free dim)
NCH = NQ // CH   # 4 chunks
JT = N // P      # 32 key tiles
SUBS = CH // P   # 4 i-subtiles per chunk
NROW = 4         # QK row-tiling factor (128/D)
NG = JT // NROW  # 8 jt-groups per chunk
NSLAB = 8        # xf DMA slabs (overlap load with kproj)

_NC_CACHE = []
_NEFF_CACHE = {}
_ONES = np.ones((P, JT, 2), ml_dtypes.bfloat16)


def _precompile(nc):
    """Compile the NEFF at plain-Python level (the in-jax XLA-callback
    compile path deadlocks on large fp32 kernels) and serve it to
    bass2jax's neuronx_cc_hook from a cache keyed on the BIR bytes."""
    from concourse.bass_utils import compile_bass_kernel
    bir = nc.to_json_bytes()
    key = hashlib.sha256(bir).hexdigest()
    if key not in _NEFF_CACHE:
        td = tempfile.mkdtemp(prefix="kneff_")
        neff = compile_bass_kernel(nc, td)
        with open(neff, "rb") as f:
            _NEFF_CACHE[key] = f.read()

    import concourse.bass2jax as b2j
    if not getattr(b2j, "_attn_kernel_neff_patch", False):
        orig = b2j.compile_bir_kernel

        def patched(bir_json, tmpdir, neff_name="file.neff"):
            bj = bir_json if isinstance(bir_json, bytes) else bir_json.encode()
            data = _NEFF_CACHE.get(hashlib.sha256(bj).hexdigest())
            if data is not None:
                p = os.path.join(tmpdir, neff_name)
                with open(p, "wb") as f:
                    f.write(data)
                return p
            return orig(bir_json, tmpdir, neff_name)

        b2j.compile_bir_kernel = patched
        b2j._attn_kernel_neff_patch = True


def _build():
    nc = bacc.Bacc("TRN2", target_bir_lowering=False)

    xf = nc.dram_tensor("xf", [P, CT, N], F16, kind="ExternalInput")
    qx = nc.dram_tensor("qx", [P, CT, NQ], F16, kind="ExternalInput")
    resid = nc.dram_tensor("resid", [P, NQ // P, C], F16, kind="ExternalInput")
    wqt = nc.dram_tensor("wqt", [P, CT, P], F16, kind="ExternalInput")
    wkt = nc.dram_tensor("wkt", [P, CT, P], F16, kind="ExternalInput")
    wvt = nc.dram_tensor("wvt", [P, CT, C], F16, kind="ExternalInput")
    bqr = nc.dram_tensor("bqr", [P, 1], F32, kind="ExternalInput")
    bkr = nc.dram_tensor("bkr", [P, 1], F32, kind="ExternalInput")
    vones = nc.dram_tensor("vones", [P, JT, 2], BF16, kind="ExternalInput")
    y = nc.dram_tensor("y", [NQ // P, P, C], F32, kind="ExternalOutput")
    DEBUG = bool(int(os.environ.get("ATTN_KERNEL_DEBUG", "0")))
    if DEBUG:
        dbg_k = nc.dram_tensor("dbg_k", [P, N // CH, CH], F16, kind="ExternalOutput")
        dbg_q = nc.dram_tensor("dbg_q", [P, NCH, CH], F16, kind="ExternalOutput")
        dbg_vt = nc.dram_tensor("dbg_vt", [P, JT, C + 2], BF16, kind="ExternalOutput")
        dbg_et = nc.dram_tensor("dbg_et", [P, 2, CH], BF16, kind="ExternalOutput")
        dbg_et2 = nc.dram_tensor("dbg_et2", [P, 2, CH], BF16, kind="ExternalOutput")
        dbg_pv = nc.dram_tensor("dbg_pv", [P, C + 2], F32, kind="ExternalOutput")

    ADD = mybir.AluOpType.add
    MUL = mybir.AluOpType.mult
    EXP = mybir.ActivationFunctionType.Exp

    with TileContext(nc) as tc:
        with (
            tc.tile_pool(name="const", bufs=1) as cp,
            tc.tile_pool(name="big", bufs=1) as bp,
            tc.tile_pool(name="expp", bufs=6) as ep,
            tc.tile_pool(name="yp", bufs=4) as yp,
            tc.tile_pool(name="rcpp", bufs=4) as rp,
            tc.tile_pool(name="psA", bufs=2, space="PSUM") as psA,
            tc.tile_pool(name="psPV", bufs=1, space="PSUM") as psPV,
        ):
            wq_sb = cp.tile([P, CT, P], F16, tag="wq")
            wk_sb = cp.tile([P, CT, P], F16, tag="wk")
            wv_sb = cp.tile([P, CT, C], F16, tag="wv")
            bq_sb = cp.tile([P, 1], F32, tag="bq")
            bk_sb = cp.tile([P, 1], F32, tag="bk")
            xf_sb = bp.tile([P, CT, N], F16, tag="xf")
            qx_sb = bp.tile([P, CT, NQ], F16, tag="qx")
            re_sb = bp.tile([P, NQ // P, C], F16, tag="resid")
            # k replicated 4x across partition strips; [P, 8, 512] so the
            # per-1024 projection DVE bias-cast has a matching 3D view
            krh_sb = bp.tile([P, N // CH, CH], F16, tag="krh")
            qrh_sb = bp.tile([P, NCH, CH], F16, tag="qrh")
            vt_sb = bp.tile([P, JT, C + 2], BF16, tag="vt")

            # split input DMA across two queues; xf slabs alternate so the
            # two queues transfer in parallel and kproj is never starved
            nsl = N // NSLAB
            nc.gpsimd.dma_start(out=wk_sb, in_=wkt[:, :, :])
            nc.gpsimd.dma_start(out=bk_sb, in_=bkr[:, :])
            nc.gpsimd.dma_start(out=wv_sb, in_=wvt[:, :, :])
            for sl in range(NSLAB):
                eng = nc.sync if sl % 2 == 0 else nc.gpsimd
                eng.dma_start(
                    out=xf_sb[:, :, sl * nsl:(sl + 1) * nsl],
                    in_=xf[:, :, sl * nsl:(sl + 1) * nsl])
            nc.gpsimd.dma_start(out=wq_sb, in_=wqt[:, :, :])
            nc.gpsimd.dma_start(out=bq_sb, in_=bqr[:, :])
            # ones column for the softmax-denominator trick
            nc.gpsimd.dma_start(out=vt_sb[:, :, C:C + 2], in_=vones[:, :, :])
            nc.gpsimd.dma_start(out=qx_sb, in_=qx[:, :, :])
            nc.sync.dma_start(out=re_sb, in_=resid[:, :, :])

            # ---- PE warmup: ~7.3us of dummy matmuls on a zeroed tile keep
            # the PE busy from t~7us so the HAM clock gate is guaranteed to
            # flip to 8/8 (2.4GHz; needs one fully-busy free-running 3.4us
            # window) before the real projections start. Input DMA lands
            # ~10-18us, so this costs little wall-clock and makes the whole
            # projection phase run at 2.4GHz instead of 1.2GHz.
            zw = cp.tile([P, CH], F16, tag="zwarm")
            nc.vector.memset(zw[:, :], 0)
            for w in range(10):
                ps = psA.tile([P, 2, CH], F32, tag="qk")
                for i in range(4):
                    nc.tensor.matmul(
                        ps[:, i // 2, (i % 2) * P:(i % 2) * P + P],
                        zw[:, 0:P], zw[:, 0:P], start=True, stop=True)

            # ---- projections. kproj (k replicated on 4 partition strips
            # via weight col-tiling) is interleaved with vproj per xf slab
            # pair so the PE tracks DMA arrival; qproj (qx lands last) runs
            # at the end.
            def kq_proj(w_sb, b_sb, src_sb, dst, g):
                ps = psA.tile([P, 2, CH], F32, tag="qk")
                for i in range(2):
                    chn = 2 * g + i
                    for ct in range(CT):
                        nc.tensor.matmul(
                            ps[:, i, :],
                            w_sb[:, ct, :],
                            src_sb[:, ct, chn * CH:(chn + 1) * CH],
                            start=(ct == 0),
                            stop=(ct == CT - 1),
                        )
                nc.vector.tensor_scalar(
                    out=dst[:, 2 * g:2 * g + 2, :], in0=ps[:, :, :],
                    scalar1=b_sb[:, :], scalar2=None, op0=ADD)

            def vproj(jg):
                ps = psA.tile([P, 4, C], F32, tag="qk")
                for jtl in range(4):
                    jt = 4 * jg + jtl
                    for ct in range(CT):
                        nc.tensor.matmul(
                            ps[:, jtl, :],
                            xf_sb[:, ct, jt * P:(jt + 1) * P],
                            wv_sb[:, ct, :],
                            start=(ct == 0),
                            stop=(ct == CT - 1),
                        )
                if jg % 2 == 0:
                    nc.scalar.copy(out=vt_sb[:, 4 * jg:4 * jg + 4, 0:C],
                                   in_=ps[:, :, :])
                else:
                    nc.vector.tensor_copy(out=vt_sb[:, 4 * jg:4 * jg + 4, 0:C],
                                          in_=ps[:, :, :])

            for g in range(N // CH // 2):
                vproj(2 * g)
                kq_proj(wk_sb, bk_sb, xf_sb, krh_sb, g)
                vproj(2 * g + 1)
            for g in range(NQ // CH // 2):
                kq_proj(wq_sb, bq_sb, qx_sb, qrh_sb, g)

            # ---- attention building blocks
            def qk_group(ch, g):
                # 4 row-tiled K=32 fp16 matmuls run concurrently on the
                # PE; exp over [128, 1024] (2 jt) per ACTIVATE -> bf16
                tiles = []
                for half in range(2):
                    ps = psA.tile([P, 2, CH], F32, tag="qk")
                    for i in range(2):
                        r = 2 * half + i
                        jt = NROW * g + r
                        rs = slice(32 * r, 32 * (r + 1))
                        reg, off = (jt * P) // CH, (jt * P) % CH
                        nc.tensor.matmul(
                            ps[:, i, :],
                            krh_sb[rs, reg, off:off + P],
                            qrh_sb[rs, ch, :],
                            start=True, stop=True,
                            tile_position=(32 * r, 0),
                        )
                    et = ep.tile([P, 2, CH], BF16, tag="exp")
                    nc.scalar.activation(et[:, :, :], ps[:, :, :], EXP)
                    if DEBUG and ch == 0 and g == 0:
                        nc.sync.dma_start(
                            out=(dbg_et if half == 0 else dbg_et2)[:, :, :],
                            in_=et[:, :, :])
                    tiles.append(et)
                return tiles

            def pv_group(g, tiles, pv):
                for r in range(NROW):
                    jt = NROW * g + r
                    et = tiles[r // 2]
                    for s in range(SUBS):
                        nc.tensor.matmul(
                            pv[s][:, :],
                            et[:, r % 2, s * P:(s + 1) * P],
                            vt_sb[:, jt, :],
                            start=(jt == 0),
                            stop=(jt == JT - 1),
                        )

            def normalize(ch, pv):
                for s in range(SUBS):
                    t = ch * SUBS + s
                    if DEBUG and ch == 0 and s == 0:
                        pvdbg = yp.tile([P, C + 2], F32, tag="yt", name="pvdbg")
                        nc.vector.tensor_copy(out=pvdbg, in_=pv[s][:, :])
                        nc.sync.dma_start(out=dbg_pv[:, :], in_=pvdbg)
                    rc = rp.tile([P, 1], F32, tag="rc")
                    nc.vector.reciprocal(rc[:, :], pv[s][:, C:C + 1])
                    yt = yp.tile([P, C], F32, tag="yt")
                    nc.vector.tensor_scalar(
                        out=yt[:, :], in0=pv[s][:, 0:C],
                        scalar1=rc[:, :], scalar2=None, op0=MUL)
                    nc.vector.tensor_tensor(
                        out=yt[:, :], in0=yt[:, :], in1=re_sb[:, t, :], op=ADD)
                    nc.gpsimd.dma_start(out=y[t, :, :], in_=yt[:, :])

            # pre-issue QK(0,0) so ScalarE has exp work during vproj
            pvs = {0: [psPV.tile([P, C + 2], F32, tag=f"pv{s}", name=f"pv{s}")
                       for s in range(SUBS)]}
            pending = (0, 0, qk_group(0, 0))

            if DEBUG:
                nc.sync.dma_start(out=dbg_k[:, :, :], in_=krh_sb[:, :, :])
                nc.sync.dma_start(out=dbg_q[:, :, :], in_=qrh_sb[:, :, :])
                nc.sync.dma_start(out=dbg_vt[:, :, :], in_=vt_sb[:, :, :])

            # ---- attention: flat (chunk, group) loop, software-pipelined
            # one group ahead (including across chunk boundaries)
            steps = [(ch, g) for ch in range(NCH) for g in range(NG)]
            for ch, g in steps[1:] + [(None, None)]:
                if ch is not None:
                    if g == 0:
                        pvs[ch] = [psPV.tile([P, C + 2], F32, tag=f"pv{s}",
                                             name=f"pv{s}")
                                   for s in range(SUBS)]
                    tiles = qk_group(ch, g)
                pch, pg, ptiles = pending
                pv_group(pg, ptiles, pvs[pch])
                if pg == NG - 1:
                    normalize(pch, pvs.pop(pch))
                pending = (ch, g, tiles) if ch is not None else None

    nc.compile()
    return nc


def make_in_maps(x, Wq, bq, Wk, bk, Wv, bv):
    x = np.ascontiguousarray(x, np.float32)
    Wq = np.asarray(Wq, np.float32)
    bq = np.asarray(bq, np.float32)
    Wk = np.asarray(Wk, np.float32)
    bk = np.asarray(bk, np.float32)
    Wv = np.asarray(Wv, np.float32)
    bv = np.asarray(bv, np.float32)

    wqt = np.ascontiguousarray(
        np.tile(Wq.T.reshape(CT, P, D).transpose(1, 0, 2), (1, 1, NROW))
    ).astype(np.float16)
    wkt = np.ascontiguousarray(
        np.tile(Wk.T.reshape(CT, P, D).transpose(1, 0, 2), (1, 1, NROW))
    ).astype(np.float16)
    wvt = np.ascontiguousarray(
        Wv.T.reshape(CT, P, C).transpose(1, 0, 2)).astype(np.float16)
    bqr = np.ascontiguousarray(np.tile(bq, NROW).reshape(P, 1))
    bkr = np.ascontiguousarray(np.tile(bk, NROW).reshape(P, 1))

    in_maps = []
    for core in range(8):
        b, h = divmod(core, 2)
        xb = x[b].reshape(C, N)
        xf_h = np.ascontiguousarray(
            xb.reshape(CT, P, N).transpose(1, 0, 2)).astype(np.float16)
        qx_h = np.ascontiguousarray(xf_h[:, :, h * NQ:(h + 1) * NQ])
        res_h = np.ascontiguousarray(
            (xb[:, h * NQ:(h + 1) * NQ].T + bv[None, :])
            .reshape(NQ // P, P, C).transpose(1, 0, 2)).astype(np.float16)
        in_maps.append({
            "xf": xf_h, "qx": qx_h, "resid": res_h,
            "wqt": wqt, "wkt": wkt, "wvt": wvt,
            "bqr": bqr, "bkr": bkr,
            "vones": _ONES,
        })
    return in_maps


def kernel(x, Wq, bq, Wk, bk, Wv, bv):
    if not _NC_CACHE:
        _NC_CACHE.append(_build())
    nc = _NC_CACHE[0]
    _precompile(nc)

    in_maps = make_in_maps(x, Wq, bq, Wk, bk, Wv, bv)
    res = run_bass_kernel_spmd(nc, in_maps, core_ids=list(range(8)))

    B = np.asarray(x).shape[0]
    out = np.empty((B, C, N), np.float32)
    for core in range(8):
        b, h = divmod(core, 2)
        slab = res.results[core]["y"].reshape(NQ, C)
        out[b, :, h * NQ:(h + 1) * NQ] = slab.T
    return out.reshape(B, C, 64, 64)
